# revision 17
# baseline (speedup 1.0000x reference)
"""LSEP loss kernel for Trainium2 (8 NeuronCores, SPMD data-parallel).

loss = log1p( sum_i [ (sum_{c: t=0} exp(x_ic)) * (sum_{c: t=1} exp(-x_ic)) ] )

Strategy (v6, per-chunk mixed a/b forms):  shard batch across 8 cores
(4096 rows each); partition p holds samples [32p, 32p+32) contiguous.
Every 2-sample chunk processes ONE sample in each form, so the ScalarE
(ACT) and DVE loads are balanced inside every chunk period (no phase
alternation -> no cross-engine convoys):

a-form (ScalarE: 2 exps; masked-exp trick):
  a_k = x - 50*t                     (DVE STT, f32, per sample)
  s_neg_k = sum exp(a)               (ACT, accum_out)
  s_pos_k = sum exp(-a - 50)         (ACT, scale=-1 bias=-50)

b-form (DVE: masked accumulate; exact sign-flip):
  b_k = x ^ (t << 31) = (-1)^t * x   (DVE STT, i32 bit domain, per sample)
  u_k  = sum exp(b)                  (ACT, accum_out; e kept bf16)
  sp_k = sum (t*1.0) * e             (DVE STT, accum_out, emitted 1 chunk late)
  s_neg_k = u_k - sp_k               (epilogue)

Per 2-sample chunk (DMA period ~4.7us @ ~420GB/s): DVE ~3.7us, ACT ~4.2us.
Ramp singles lean b-form (short first chain), tail singles a-form (shortest
drain chain).  Epilogue: prod = s_neg*s_pos, reduce, DMA [128,1] partial;
host sums 1024x8 partials and applies log1p.

HW facts (measured via ntff traces): DVE 1 elem/lane/cycle @0.96GHz for all
dtypes; ACT ~1128ns/[128,1000] exp + 278ns accumulator read; GpSimd can't
run TensorScalarPtr; DMA ~420 GB/s steady => 32.77MB/core ~ 78us floor.
"""

import numpy as np

BATCH = 32768
C = 1000
N_CORES = 8
ROWS = BATCH // N_CORES          # 4096 rows per core
P = 128                          # SBUF partitions
SPR = ROWS // P                  # 32 samples per partition
# small chunks at the ramp for fast pipeline start; a-single tail drains fast
CHUNKS = [1] * 5 + [2] * 13 + [1]  # sum == 32
# 14 a / 18 b samples: ramp singles b-form, middle chunks mixed (a,b),
# tail single a-form.  At most ONE b-sample per chunk (they share et[:,0:C]).
def _default_forms():
    forms = []
    for ci, ncols in enumerate(CHUNKS):
        if ncols == 1:
            forms.append("b" if ci < 5 else "a")
        else:
            forms.append("ab")
    return forms

_CACHE = {}


def _build_nc():
    import concourse.bacc as bacc
    import concourse.mybir as mybir
    from concourse.tile import TileContext

    f32 = mybir.dt.float32
    bf16 = mybir.dt.bfloat16
    i32 = mybir.dt.int32
    Exp = mybir.ActivationFunctionType.Exp
    Alu = mybir.AluOpType

    assert sum(CHUNKS) == SPR
    wmax = max(CHUNKS) * C
    forms = _default_forms()

    nc = bacc.Bacc()
    x = nc.declare_dram_parameter("input", [ROWS, C], i32, isOutput=False)
    t = nc.declare_dram_parameter("target", [ROWS, C], i32, isOutput=False)
    out = nc.declare_dram_parameter("partial", [P, 1], f32, isOutput=True)

    xv = x.rearrange("(p s) c -> p (s c)", p=P)
    tv = t.rearrange("(p s) c -> p (s c)", p=P)

    def stt_shift_xor(out_ap, t_ap, x_ap):
        # b = (t << 31) ^ x.  walrus birverifier requires bitvec-op
        # immediates to be integer-typed and match src/dst dtype.
        eng = nc.vector
        eng.add_instruction(
            mybir.InstTensorScalarPtr(
                name=nc.get_next_instruction_name(),
                is_scalar_tensor_tensor=True,
                op0=Alu.logical_shift_left,
                op1=Alu.bitwise_xor,
                ins=[
                    eng.lower_ap(t_ap),
                    mybir.ImmediateValue(dtype=i32, value=31),
                    eng.lower_ap(x_ap),
                ],
                outs=[eng.lower_ap(out_ap)],
            )
        )

    with TileContext(nc) as tc:
        with (
            tc.tile_pool(name="io", bufs=10) as io,
            tc.tile_pool(name="acc", bufs=1) as accp,
        ):
            sn = accp.tile([P, SPR], f32)     # s_neg (a-form) / u (b-form)
            sta = accp.tile([P, SPR], f32)    # s_pos, a-form cols (ACT writes)
            stb = accp.tile([P, SPR], f32)    # s_pos, b-form cols (DVE writes)
            scr_a = accp.tile([P, C], bf16)   # discarded ACT#2 main out
            scr_s = accp.tile([P, C], bf16)   # discarded sp-accum main out
            bneg = accp.tile([P, 1], f32)     # bias AP holding -50.0
            nc.vector.memset(bneg[:], -50.0)

            def emit_sp(pend):
                for k, tt_s, esl in pend:
                    nc.vector.scalar_tensor_tensor(
                        scr_s[:], tt_s, 1.0, esl,
                        op0=Alu.mult, op1=Alu.mult,
                        accum_out=stb[:, k : k + 1],
                    )

            pending = []  # deferred per-chunk sp-accum args
            ia = SPR  # a-form accum columns, filled right-to-left
            ib = 0    # b-form accum columns, filled left-to-right
            off = 0
            for ci, ncols in enumerate(CHUNKS):
                w = ncols * C
                xt = io.tile([P, wmax], i32, tag="x")
                tt = io.tile([P, wmax], i32, tag="t")
                if "b" in forms[ci]:
                    et = io.tile([P, C], bf16, tag="e", name=f"et_{ci}")
                else:
                    et = None
                nc.sync.dma_start(tt[:, :w], tv[:, off * C : off * C + w])
                nc.sync.dma_start(xt[:, :w], xv[:, off * C : off * C + w])
                chunk_pend = []
                for j, f in enumerate(forms[ci]):
                    lo, hi = j * C, (j + 1) * C
                    if f == "a":
                        ia -= 1
                        k = ia
                        asl = xt[:, lo:hi].bitcast(f32)
                        nc.vector.scalar_tensor_tensor(
                            asl, tt[:, lo:hi], -50.0, asl,
                            op0=Alu.mult, op1=Alu.add,
                        )
                        nc.scalar.activation(
                            scr_a[:], asl, Exp, accum_out=sn[:, k : k + 1]
                        )
                        nc.scalar.activation(
                            scr_a[:], asl, Exp, scale=-1.0, bias=bneg[:],
                            accum_out=sta[:, k : k + 1],
                        )
                    else:
                        k = ib
                        ib += 1
                        stt_shift_xor(xt[:, lo:hi], tt[:, lo:hi], xt[:, lo:hi])
                        esl = et[:, 0:C]
                        nc.scalar.activation(
                            esl, xt[:, lo:hi].bitcast(f32), Exp,
                            accum_out=sn[:, k : k + 1],
                        )
                        chunk_pend.append((k, tt[:, lo:hi], esl))
                if chunk_pend:
                    pending.append(chunk_pend)
                # flush sp-accums three chunks late: coarse (emission-order)
                # cross-engine semaphore waits are then already satisfied, so
                # DVE never stalls on a fresh ACT output
                if len(pending) > 3:
                    emit_sp(pending.pop(0))
                off += ncols
            for chunk_pend in pending:
                emit_sp(chunk_pend)

            # epilogue: b-cols are [0:ib): s_neg = u - sp, prod = s_neg*sp;
            # a-cols are [ib:SPR): prod = sn*sta.  One contiguous op each.
            pr = accp.tile([P, SPR], f32)
            d = accp.tile([P, SPR], f32)
            tot = accp.tile([P, 1], f32)
            assert ia == ib
            nb = ib
            nc.vector.tensor_tensor(
                d[:, 0:nb], sn[:, 0:nb], stb[:, 0:nb], Alu.subtract
            )
            nc.vector.tensor_tensor(
                pr[:, 0:nb], d[:, 0:nb], stb[:, 0:nb], Alu.mult
            )
            nc.vector.tensor_tensor(
                pr[:, nb:SPR], sn[:, nb:SPR], sta[:, nb:SPR], Alu.mult
            )
            nc.vector.reduce_sum(tot[:], pr[:], axis=mybir.AxisListType.X)
            # out-DMA on the ACT HWDGE ring: the sync ring's FIFO still
            # holds input-DMA completions at this point
            nc.scalar.dma_start(out[:], tot[:])
    nc.compile()
    return nc


def _get_nc():
    if "nc" not in _CACHE:
        _CACHE["nc"] = _build_nc()
    return _CACHE["nc"]


def kernel(input, target):
    from concourse.bass_utils import run_bass_kernel_spmd

    x = np.ascontiguousarray(np.asarray(input, dtype=np.float32))
    t = np.ascontiguousarray(np.asarray(target, dtype=np.int32))
    assert x.shape == (BATCH, C) and t.shape == (BATCH, C)
    xi = x.view(np.int32)   # raw-bits view; b-form flips the sign bit via xor

    nc = _get_nc()
    in_maps = [
        {
            "input": xi[i * ROWS : (i + 1) * ROWS],
            "target": t[i * ROWS : (i + 1) * ROWS],
        }
        for i in range(N_CORES)
    ]
    res = run_bass_kernel_spmd(nc, in_maps, list(range(N_CORES)))
    total = 0.0
    for r in res.results:
        total += float(np.sum(r["partial"].astype(np.float64)))
    return np.asarray([np.log1p(total)], dtype=np.float32)
